# revision 1
# baseline (speedup 1.0000x reference)
"""LDPC encoder kernel for Trainium2 (8 NeuronCores, batch-sharded).

Computes out = 1 - 2*((m @ G^T) mod 2)  (BPSK-mapped LDPC codeword).

  m: [16384, 1200] int32 (0/1)   G: [2400, 1200] float32 (0/1)
  out: [16384, 2400] float32 (+-1)

Strategy:
  - Shard the batch over 8 cores (2048 rows each); G replicated.
  - G is systematic (G[:1200] == I), so out[:, :1200] = 1 - 2*m is a pure
    elementwise map; only the 1200 parity columns need a matmul.
  - Matmul in bf16 (values 0/1/2 are exact; PSUM accumulates fp32 exactly).
    Host feeds m transposed ([K,B] layout) so the stationary operand needs
    no on-device transpose, plus G^T scaled by 2 with an extra all-ones/2
    bias row so PSUM holds 2*d + 2. Then a single DVE op per tile:
        out = (psum mod 4) - 1  ->  {+1 even d, -1 odd d}
  - Output written as bf16 (+-1 exact), cast to f32 on host.
"""

import numpy as np
import ml_dtypes

BF16 = ml_dtypes.bfloat16

B_FULL = 16384
K_MSG = 1200
N_BITS = 2400
N_CORES = 8
B_LOC = B_FULL // N_CORES  # 2048
K_PAD = 1280  # 10 k-tiles of 128; row 1200 is the +2 bias row
P = 128

_CACHE: dict = {}
# fp8 DoubleRow matmul (2 contraction rows per PE cell): compiles and is
# exact in CoreSim, but the generated NEFF hit NRT_EXEC_UNIT_UNRECOVERABLE
# on hardware — keep the proven bf16 path.
USE_DR = False


def _mm_np_dtype():
    if not USE_DR:
        return BF16
    import concourse.mybir as mybir
    return mybir.dt.np(mybir.dt.float8e4)


def _build(bl, k_msg, k_pad, n_par, n_bits, base_col, with_identity,
           use_dr=False):
    """Build + compile the per-core Bass program.

    bl: local batch rows; n_par: matmul output columns; base_col: where the
    matmul columns land in the output; with_identity: also emit
    out[:, :k_msg] = 1-2*m from a natural-layout copy of m.
    """
    import concourse.bacc as bacc
    import concourse.mybir as mybir
    import concourse.tile as tile

    bf16 = mybir.dt.bfloat16
    f32 = mybir.dt.float32
    i32 = mybir.dt.int32
    Alu = mybir.AluOpType
    Act = mybir.ActivationFunctionType

    nc = bacc.Bacc("TRN2", target_bir_lowering=False, debug=False,
                   num_devices=N_CORES)

    fp8 = mybir.dt.float8e4
    mm_dt = fp8 if use_dr else bf16
    mT = nc.dram_tensor("mT", [k_pad, bl], mm_dt, kind="ExternalInput")
    gT = nc.dram_tensor("GT2", [k_pad, n_par], mm_dt, kind="ExternalInput")
    out = nc.dram_tensor("out", [bl, n_bits], bf16, kind="ExternalOutput")
    mnat = None
    if with_identity:
        mnat = nc.dram_tensor("mnat", [bl, k_msg], bf16, kind="ExternalInput")

    k_step = 2 * P if use_dr else P
    kt_n = k_pad // k_step
    nb = bl // P
    chunks = []
    n0 = 0
    while n0 < n_par:
        w = min(512, n_par - n0)
        chunks.append((n0, w))
        n0 += w

    with tile.TileContext(nc) as tc:
        with (
            tc.tile_pool(name="const", bufs=1) as cpool,
            tc.tile_pool(name="mn", bufs=3) as mnpool,
            tc.tile_pool(name="po", bufs=6) as popool,
            tc.tile_pool(name="io", bufs=3) as iopool,
            tc.tile_pool(name="ps", bufs=6, space="PSUM") as pspool,
        ):
            gts, mts = [], []
            for t in range(kt_n):
                ks = slice(t * k_step, (t + 1) * k_step)
                if use_dr:
                    # [2*P, X] DRAM rows -> [P, 2, X] SBUF (k = t*256 + i*128 + p)
                    gt_t = cpool.tile([P, 2, n_par], mm_dt, tag=f"gt{t}")
                    nc.sync.dma_start(
                        out=gt_t[:],
                        in_=gT[ks, :].rearrange("(i p) c -> p i c", i=2))
                    mt_t = cpool.tile([P, 2, bl], mm_dt, tag=f"mt{t}")
                    nc.sync.dma_start(
                        out=mt_t[:],
                        in_=mT[ks, :].rearrange("(i p) c -> p i c", i=2))
                else:
                    gt_t = cpool.tile([P, n_par], mm_dt, tag=f"gt{t}")
                    nc.sync.dma_start(out=gt_t[:], in_=gT[ks, :])
                    mt_t = cpool.tile([P, bl], mm_dt, tag=f"mt{t}")
                    nc.sync.dma_start(out=mt_t[:], in_=mT[ks, :])
                gts.append(gt_t)
                mts.append(mt_t)

            for b in range(nb):
                bs = slice(b * P, (b + 1) * P)
                psts = [pspool.tile([P, 512], f32, tag="ps", name=f"ps{b}_{ci}")
                        for ci in range(len(chunks))]
                for t in range(kt_n):
                    for ci, (n0, w) in enumerate(chunks):
                        if use_dr:
                            nc.tensor.matmul(
                                psts[ci][:, :w],
                                mts[t][:, :, bs],
                                gts[t][:, :, n0:n0 + w],
                                start=(t == 0),
                                stop=(t == kt_n - 1),
                                perf_mode=mybir.MatmulPerfMode.DoubleRow,
                            )
                        else:
                            nc.tensor.matmul(
                                psts[ci][:, :w],
                                mts[t][:, bs],
                                gts[t][:, n0:n0 + w],
                                start=(t == 0),
                                stop=(t == kt_n - 1),
                            )
                for ci, (n0, w) in enumerate(chunks):
                    # parity -> BPSK: p = int(d) & 1 ; out = -2p + 1
                    it = popool.tile([P, 512], i32, tag="pi",
                                     name=f"pi{b}_{ci}")
                    nc.vector.tensor_copy(it[:, :w], psts[ci][:, :w])
                    pt = popool.tile([P, 512], i32, tag="pp",
                                     name=f"pp{b}_{ci}")
                    nc.vector.tensor_scalar(
                        pt[:, :w], it[:, :w], 1, None, op0=Alu.bitwise_and,
                    )
                    ot = popool.tile([P, 512], bf16, tag="po",
                                     name=f"po{b}_{ci}")
                    nc.vector.tensor_scalar(
                        ot[:, :w], pt[:, :w], -2.0, 1.0,
                        op0=Alu.mult, op1=Alu.add,
                    )
                    nc.sync.dma_start(
                        out=out[bs, base_col + n0:base_col + n0 + w],
                        in_=ot[:, :w],
                    )
                if with_identity:
                    mn = mnpool.tile([P, k_msg], bf16, tag="mn")
                    nc.sync.dma_start(out=mn[:], in_=mnat[bs, :])
                    io = iopool.tile([P, k_msg], bf16, tag="io")
                    nc.vector.tensor_scalar(
                        io[:], mn[:], -2.0, 1.0, op0=Alu.mult, op1=Alu.add,
                    )
                    nc.sync.dma_start(out=out[bs, 0:k_msg], in_=io[:])

    nc.compile()
    return nc


def _get_nc(fast: bool):
    key = ("fast" if fast else "full", USE_DR)
    if key not in _CACHE:
        if fast:
            _CACHE[key] = _build(B_LOC, K_MSG, K_PAD, N_BITS - K_MSG, N_BITS,
                                 K_MSG, True, use_dr=USE_DR)
        else:
            _CACHE[key] = _build(B_LOC, K_MSG, K_PAD, N_BITS, N_BITS, 0, False,
                                 use_dr=USE_DR)
    return _CACHE[key]


def _prep_inputs(m, G, fast: bool):
    """Host-side marshaling: casts, transposes, padding, bias row."""
    mm_dt = _mm_np_dtype()
    m_mm = m.astype(mm_dt)
    if fast:
        g_rows = G[K_MSG:N_BITS]  # parity rows only
    else:
        g_rows = G
    n_par = g_rows.shape[0]
    gT2 = np.zeros((K_PAD, n_par), dtype=mm_dt)
    gT2[:K_MSG] = g_rows.T.astype(mm_dt)  # psum = d (count of set bits)

    in_maps = []
    for c in range(N_CORES):
        m_c = m_mm[c * B_LOC:(c + 1) * B_LOC]
        mT = np.zeros((K_PAD, B_LOC), dtype=mm_dt)
        mT[:K_MSG] = np.ascontiguousarray(m_c.T)
        im = {"mT": mT, "GT2": gT2}
        if fast:
            im["mnat"] = np.ascontiguousarray(
                m[c * B_LOC:(c + 1) * B_LOC].astype(BF16))
        in_maps.append(im)
    return in_maps


def _run(m, G, trace=False):
    from concourse.bass_utils import run_bass_kernel_spmd

    fast = bool(
        np.array_equal(G[:K_MSG], np.eye(K_MSG, dtype=G.dtype))
        and ((G == 0) | (G == 1)).all()
    )
    nc = _get_nc(fast)
    in_maps = _prep_inputs(m, G, fast)
    res = run_bass_kernel_spmd(
        nc, in_maps, core_ids=list(range(N_CORES)), trace=trace,
    )
    parts = [res.results[c]["out"] for c in range(N_CORES)]
    full = np.concatenate(parts, axis=0).astype(np.float32)
    return full, res


def kernel(m, G, snr=None):
    m = np.asarray(m)
    G = np.asarray(G)
    full, _ = _run(m, G, trace=False)
    return full



# revision 17
# speedup vs baseline: 44.7942x; 44.7942x over previous
"""LDPC encoder kernel for Trainium2 (8 NeuronCores, batch-sharded).

Computes out = 1 - 2*((m @ G^T) mod 2)  (BPSK-mapped LDPC codeword).

  m: [16384, 1200] int32 (0/1)   G: [2400, 1200] float32 (0/1)
  out: [16384, 2400] float32 (+-1)

Strategy:
  - Shard the batch over 8 cores (2048 rows each); G replicated.
  - G is systematic (G[:1200] == I), so out[:, :1200] = 1 - 2*m is a pure
    elementwise map; only the 1200 parity columns need a matmul.
  - All device I/O in fp8e4m3 (values 0/1/2 and +-1 are exact): halves DMA
    bytes vs bf16. Matmul in fp8 without a perf mode runs at bf16 speed and
    accumulates exactly into fp32 PSUM.
  - Host feeds m transposed ([K,B] layout) so the stationary operand needs
    no on-device transpose; PSUM holds d exactly. Three DVE ops per chunk
    map parity to BPSK (the DVE ISA has no mod, forbids mixing bitwise and
    arith ops in one tensor_scalar, and TRN fp8e4 tops out at +-240 so a
    2^23 matmul bias row is not buildable):
        i = int32(psum) ; p = i & 1 ; out = -2*p + 1  in {-1, +1}.
    DVE time (~3.1us per 128-row tile) hides fully under the ~5us of
    matmul per tile.
  - Each 128-row batch tile assembles its full 2400-column output row in
    one SBUF tile (identity + parity) and stores it with a single fully
    contiguous 300 KB DMA.
  - Output fp8 (+-1 exact), cast to f32 on host.
  - reps>1 wraps the body in a For_i hardware loop: used by test.py to
    measure per-execution device time by slope (amortizes dispatch RTT).
"""

import numpy as np

B_FULL = 16384
K_MSG = 1200
N_BITS = 2400
N_CORES = 8
B_LOC = B_FULL // N_CORES  # 2048
K_PAD = 1280  # 10 k-tiles of 128; rows 1200..1279 are zero padding
P = 128

_CACHE: dict = {}
# fp8 DoubleRow matmul: 2 contraction rows per PE cell -> 5 k-passes of 256
# instead of 10 of 128 (~1.4x PE throughput). Verified exact on this HW by
# an isolated probe (a previous session saw an NRT crash with DR; the
# current toolchain runs it fine).
USE_DR = True


def _np_fp8():
    import concourse.mybir as mybir
    return mybir.dt.np(mybir.dt.float8e4)


def _build(bl, n_par, base_col, with_identity, reps=1, use_dr=False):
    """Build + compile the per-core Bass program.

    bl: local batch rows; n_par: matmul output columns; base_col: where the
    matmul columns land in the output; with_identity: also emit
    out[:, :K_MSG] = 1-2*m from a natural-layout copy of m.
    """
    import concourse.bacc as bacc
    import concourse.mybir as mybir
    import concourse.tile as tile

    fp8 = mybir.dt.float8e4
    f32 = mybir.dt.float32
    i32 = mybir.dt.int32
    Alu = mybir.AluOpType

    nc = bacc.Bacc("TRN2", target_bir_lowering=False, debug=False,
                   num_devices=N_CORES)

    mT = nc.dram_tensor("mT", [K_PAD, bl], fp8, kind="ExternalInput")
    gT2 = nc.dram_tensor("GT2", [K_PAD, n_par], fp8, kind="ExternalInput")
    out = nc.dram_tensor("out", [bl, N_BITS], fp8, kind="ExternalOutput")
    mnat = None
    if with_identity:
        mnat = nc.dram_tensor("mnat", [bl, K_MSG], fp8, kind="ExternalInput")

    k_step = 2 * P if use_dr else P
    kt_n = K_PAD // k_step  # 5 with DoubleRow, 10 without
    nb = bl // P  # 16
    chunks = []
    n0 = 0
    while n0 < n_par:
        w = min(512, n_par - n0)
        chunks.append((n0, w))
        n0 += w

    with tile.TileContext(nc) as tc:
        with (
            tc.tile_pool(name="const", bufs=1) as cpool,
            tc.tile_pool(name="mn", bufs=3) as mnpool,
            tc.tile_pool(name="po", bufs=6) as popool,
            tc.tile_pool(name="ot", bufs=3) as otpool,
            tc.tile_pool(name="ps", bufs=6, space="PSUM") as pspool,
        ):
            def body(_iv=None):
                gts, mts = [], []
                for t in range(kt_n):
                    ks = slice(t * k_step, (t + 1) * k_step)
                    if use_dr:
                        # [2P, X] DRAM rows -> [P, 2, X] SBUF; the (p, i)
                        # -> k mapping just needs to agree between the two
                        # operands for the contraction to be exact.
                        gt_t = cpool.tile([P, 2, n_par], fp8, tag=f"gt{t}",
                                          name=f"gt{t}")
                        nc.sync.dma_start(
                            out=gt_t[:],
                            in_=gT2[ks, :].rearrange("(i p) c -> p i c",
                                                     i=2))
                        mt_t = cpool.tile([P, 2, bl], fp8, tag=f"mt{t}",
                                          name=f"mt{t}")
                        nc.sync.dma_start(
                            out=mt_t[:],
                            in_=mT[ks, :].rearrange("(i p) c -> p i c",
                                                    i=2))
                    else:
                        gt_t = cpool.tile([P, n_par], fp8, tag=f"gt{t}",
                                          name=f"gt{t}")
                        nc.sync.dma_start(out=gt_t[:], in_=gT2[ks, :])
                        mt_t = cpool.tile([P, bl], fp8, tag=f"mt{t}",
                                          name=f"mt{t}")
                        nc.sync.dma_start(out=mt_t[:], in_=mT[ks, :])
                    gts.append(gt_t)
                    mts.append(mt_t)

                for b in range(nb):
                    bs = slice(b * P, (b + 1) * P)
                    ot = otpool.tile([P, N_BITS], fp8, tag="ot",
                                     name=f"ot{b}")
                    if with_identity:
                        mn = mnpool.tile([P, K_MSG], fp8, tag="mn",
                                         name=f"mn{b}")
                        nc.sync.dma_start(out=mn[:], in_=mnat[bs, :])
                    psts = [pspool.tile([P, 512], f32, tag="ps",
                                        name=f"ps{b}_{ci}")
                            for ci in range(len(chunks))]
                    for t in range(kt_n):
                        for ci, (n0, w) in enumerate(chunks):
                            if use_dr:
                                nc.tensor.matmul(
                                    psts[ci][:, :w],
                                    mts[t][:, :, bs],
                                    gts[t][:, :, n0:n0 + w],
                                    start=(t == 0),
                                    stop=(t == kt_n - 1),
                                    perf_mode=(
                                        mybir.MatmulPerfMode.DoubleRow),
                                )
                            else:
                                nc.tensor.matmul(
                                    psts[ci][:, :w],
                                    mts[t][:, bs],
                                    gts[t][:, n0:n0 + w],
                                    start=(t == 0),
                                    stop=(t == kt_n - 1),
                                )
                    if with_identity:
                        # systematic bits: out = -2*m + 1
                        nc.vector.tensor_scalar(
                            ot[:, 0:K_MSG], mn[:], -2.0, 1.0,
                            op0=Alu.mult, op1=Alu.add,
                        )
                    for ci, (n0, w) in enumerate(chunks):
                        # parity -> BPSK: p = int(d) & 1 ; out = -2p + 1
                        it = popool.tile([P, 512], i32, tag="pi",
                                         name=f"pi{b}_{ci}")
                        nc.vector.tensor_copy(it[:, :w], psts[ci][:, :w])
                        pt = popool.tile([P, 512], i32, tag="pp",
                                         name=f"pp{b}_{ci}")
                        nc.vector.tensor_scalar(
                            pt[:, :w], it[:, :w], 1, None,
                            op0=Alu.bitwise_and,
                        )
                        nc.vector.tensor_scalar(
                            ot[:, base_col + n0:base_col + n0 + w],
                            pt[:, :w], -2.0, 1.0,
                            op0=Alu.mult, op1=Alu.add,
                        )
                    nc.sync.dma_start(out=out[bs, :], in_=ot[:])

            if reps == 1:
                body()
            else:
                with tc.For_i(0, reps, 1,
                              hint_engines=(mybir.EngineType.PE,)) as _i:
                    body(_i)

    nc.compile()
    return nc


def _get_nc(fast: bool, reps: int = 1):
    key = ("fast" if fast else "full", reps)
    if key not in _CACHE:
        if fast:
            _CACHE[key] = _build(B_LOC, N_BITS - K_MSG, K_MSG, True,
                                 reps=reps)
        else:
            _CACHE[key] = _build(B_LOC, N_BITS, 0, False, reps=reps)
    return _CACHE[key]


def _prep_inputs(m, G, fast: bool):
    """Host-side marshaling: casts, transposes, padding, bias row."""
    fp8 = _np_fp8()
    g_rows = G[K_MSG:N_BITS] if fast else G  # parity rows only on fast path
    n_par = g_rows.shape[0]
    gT2 = np.zeros((K_PAD, n_par), dtype=fp8)
    gT2[:K_MSG] = g_rows.T.astype(fp8)  # psum accumulates d

    mT_full = np.zeros((K_PAD, B_FULL), dtype=fp8)
    mT_full[:K_MSG] = m.astype(np.float32).astype(fp8).T
    m8 = m.astype(np.float32).astype(fp8) if fast else None

    in_maps = []
    for c in range(N_CORES):
        cs = slice(c * B_LOC, (c + 1) * B_LOC)
        im = {"mT": np.ascontiguousarray(mT_full[:, cs]), "GT2": gT2}
        if fast:
            im["mnat"] = m8[cs]
        in_maps.append(im)
    return in_maps


def _is_fast(G):
    return bool(
        np.array_equal(G[:K_MSG], np.eye(K_MSG, dtype=G.dtype))
        and ((G == 0) | (G == 1)).all()
    )


def _run(m, G, trace=False):
    from concourse.bass_utils import run_bass_kernel_spmd

    fast = _is_fast(G)
    nc = _get_nc(fast)
    in_maps = _prep_inputs(m, G, fast)
    res = run_bass_kernel_spmd(
        nc, in_maps, core_ids=list(range(N_CORES)), trace=trace,
    )
    parts = [res.results[c]["out"] for c in range(N_CORES)]
    full = np.concatenate(parts, axis=0).astype(np.float32)
    return full, res


def kernel(m, G, snr=None):
    m = np.asarray(m)
    G = np.asarray(G)
    full, _ = _run(m, G, trace=False)
    return full
